# revision 15
# baseline (speedup 1.0000x reference)
"""DialogSeqAttnMatch Trainium2 kernel (8-core SPMD, L1-sharded).

Math (reference):
  dlg   = concat(xq, xa) reshaped (B*M, H); M = LQ+LA
  x_proj = relu(xd @ W.T + b);  y_proj = relu(dlg @ W.T + b)
  scores[b,l,k] = x_proj[b,l] . y_proj[k]  masked (causal: ts(k) >= b, padding)
                  + rw0*|b - ts(k)|  (row 0 zeroed)
  out = softmax_k(scores) @ dlg   (row 0 of alpha zeroed -> out[0] = 0)

Key techniques:
  - Recency bias separates in the causal region: exp(s + rw0*b - rw0*ts) =
    exp(s) * e^{rw0*b} * e^{-rw0*ts}; the row factor cancels in softmax, the
    column factor phi_k folds into the value rows on the host.
  - Padding: phi-scaled value rows and the appended ones-column are zeroed on
    the host, removing masked tokens from numerator and denominator.
  - Causal mask: per (batch, k-chunk) memsets of the exp'd tile (64-aligned).
  - exp engine split: chunks 0-7 (k < 1024) exp'd exactly on ACT
    (bias=-S0); chunks 8-15 exp'd on DVE via a custom 8-stage op
    implementing a quadratic-corrected Schraudolph: the bf16 bit pattern of
    e^s is ~ A*s + kappa*(w^2-4096), w = centered frac of A*s on the 128-wide
    bf16 cell (A = 128/ln2).  Max rel err ~0.6%.  The missing bits-bias
    (16256 - A*S0) is equivalent to a constant factor e^{127 ln2 - S0},
    divided out of the chunk>=8 value rows on the host.  The A-scale rides
    for free on a second weight copy A*W used for y_proj cols 1024:2048
    (those columns feed only chunk>=8 scores).
  - Normalization on host: the kernel outputs raw [numerator | denominator]
    PSUM tiles; the host divides in float64.
  - PE p-state warmup matmuls + dummy ACT exp at t=0 (hoists the 1.28us
    activation-table load off the critical path).

Device layout (per core, l-slice of 64 rows for all 32 batches):
  xdT   (128 d, 2048 (b,l))  f32r    scores computed TRANSPOSED: (k, l)
  dlgT  (128 d, 2048 k)      f32r
  x_projT = relu(Wt.T @ xdT + b)            : (128, 2048) f32r
  y_projT[:, :1024]  = relu(Wt.T @ dlgT + b); [:, 1024:] uses A*Wt, A*b
  group g (4 batches, 256 l-cols), chunks of 128 k; per chunk:
    scoresT psum (128k, 256l) = y_projT_chunk^T @ x_projT_group  [fp32r]
    pT = exp-ish (ACT exp | DVE custom) -> bf16, stacked 4 chunks/instr
    out_psum(A/B) (128 l, 129) += pT_half^T @ dlg_aug_chunk      [bf16]
  flush: copy psum (128, 260) -> SBUF -> DMA out[g]; host normalizes.
Group order [0,1,2,3,4,5,7,6] puts a 2-chunk stack last (short tail).
"""
import os
import sys

sys.path.insert(0, "/opt/trn_rl_repo")

import numpy as np
import ml_dtypes

import concourse.bass as bass
import concourse.tile as tile
import concourse.mybir as mybir
from concourse import bacc
from concourse import dve_ops as dvo
from concourse.bass_utils import run_bass_kernel_spmd
from concourse.dve_spec import Spec, Src0, C0, C1, C2, sq, Latch, lower
from concourse.dve_uop import DveOpSpec

F32 = mybir.dt.float32
F32R = mybir.dt.float32r
BF16 = mybir.dt.bfloat16
I16 = mybir.dt.int16

B, L1, LQ, LA, H = 32, 512, 32, 32, 128
M = LQ + LA              # 64 tokens per timestep
K = B * M                # 2048 flattened history
NCORES = 8
LC = L1 // NCORES        # 64 l-rows per core
S0 = 40.0                # exp shift (baseline scale for ACT chunks)
T0 = 16.0                # phi centering
NG = 8                   # batch groups of 4 (256 l-cols each)
STACK = 4                # k-chunks stacked per PSUM buffer / exp instruction
ORDER = [0, 1, 2, 3, 4, 5, 7, 6]   # processing order; 2-chunk stack last

A_C = 128.0 / np.log(2.0)          # bf16 bits per natural-log unit
M128 = float(3 * (2 ** 22) * 128)  # f32 ulp here = 128 (one bf16 cell)
KAPPA = 2.625e-3                   # quadratic mantissa correction
# chunk >= 8 values carry e^{127 ln2 - S0} so DVE bits (= A*s) match ACT's
# exp(s - S0) after the softmax divide.
LOG_RDIV = 127.0 * np.log(2.0) - S0

_OPNAME = "SCHRAUD_EXP2_ANT"
_NC_CACHE = None


def _register_exp_op():
    """Custom DVE op: out = Z + kappa*(w^2 - 4096), w = centered frac of Z
    on the 128-wide bf16 cell.  Written through an int16 view, the result
    bits decode (as bf16) to ~e^{Z/A} with ~0.6% max error."""
    if _OPNAME in dvo._SUB_OPCODE_FOR_NAME:
        return next(o for o in dvo.OPS if o.name == _OPNAME)
    dvo._SUB_OPCODE_FOR_NAME[_OPNAME] = max(dvo._SUB_OPCODE_FOR_NAME.values()) + 1

    def _ref(in0, in1, c0, c1, c2):
        z = in0.astype(np.float32)
        a = (z - np.float32(c0)).astype(np.float32)
        u = (a + np.float32(c1)).astype(np.float32)
        v = (u - np.float32(c1)).astype(np.float32)
        w = (a - v).astype(np.float32)
        w2c = (w * w - np.float32(c0) * np.float32(c0)).astype(np.float32)
        return (z + w2c * np.float32(c2)).astype(np.float32)

    a_n = Src0 - C0
    w_n = a_n - ((a_n + C1) - C1)
    spec = Spec(body=Src0 + (sq(w_n) - Latch(C0 * C0)) * C2, reference=_ref)
    sha = DveOpSpec(name=_OPNAME, opcode=dvo.get_dve_sub_opcode(_OPNAME),
                    uops=lower(spec, ver="v3"), rd1_en=False).sha("v3")
    op = dvo.DveOp(_OPNAME, spec, subdim=False, uops_sha={"v3": sha})
    dvo.OPS.append(op)
    dvo.CUSTOM_DVE_SPECS[_OPNAME] = spec
    return op


EXP_OP = _register_exp_op()


def _chunks_of_group(g):
    return 2 * g + 2


def _build():
    nc = bacc.Bacc("TRN2", target_bir_lowering=False, debug=False)

    # host-packed inputs (DMA'd piece-wise in consumption order):
    #   inp1 = [Wt | bcol | Wt2=A*Wt | bcol2=A*b | dlgT]   f32r-rounded
    #   inp2 = xdT (d x (b,l)) for this core's l-slice      f32r-rounded
    #   inp3 = dlg_aug (phi*, pad-zeroed, chunk>=8 un-biased) chunk-tiled bf16
    inp1 = nc.dram_tensor("inp1", [H, 258 + K], F32R,
                          kind="ExternalInput").ap()
    inp2 = nc.dram_tensor("inp2", [H, B * LC], F32R, kind="ExternalInput").ap()
    inp3 = nc.dram_tensor("inp3", [128, 16 * 129], BF16, kind="ExternalInput").ap()

    # raw per-group [psA | gap | psB] tiles; host normalizes + reorders
    out = nc.dram_tensor("out", [NG, 128, 260], F32, kind="ExternalOutput").ap()

    with tile.TileContext(nc) as tc:
        with tc.tile_pool(name="const", bufs=1) as cpool, \
             tc.tile_pool(name="pt", bufs=8) as ptpool, \
             tc.tile_pool(name="osb", bufs=3) as osbpool, \
             tc.tile_pool(name="ps_big", bufs=3, space="PSUM") as psb, \
             tc.tile_pool(name="ps_out", bufs=2, space="PSUM") as pso:

            # --- t=0: PE p-state warmup + ACT table-load hoist ---
            negs0 = cpool.tile([128, 1], F32)
            nc.vector.memset(negs0[:], -S0)
            # bias matching the DVE bit-convention: e^{s - S0 - LOG_RDIV},
            # for chunk>=8 stacks run on ACT (input arrives A-scaled).
            negs2 = cpool.tile([128, 1], F32)
            nc.vector.memset(negs2[:], -(S0 + float(LOG_RDIV)))
            warm = cpool.tile([128, 256], BF16)
            nc.gpsimd.memset(warm[:], 1.0)
            dummy = cpool.tile([128, 1], F32)
            nc.scalar.activation(dummy[:], negs0[:],
                                 mybir.ActivationFunctionType.Exp,
                                 bias=0.0, scale=1.0)
            # junk matmuls keep the PE p-state ramp alive through the DMA
            # prologue so the first real matmuls run at full clock.
            for wi in range(8):
                wps = psb.tile([128, STACK * 256], F32, tag="scps",
                               name=f"warmps{wi}")
                nc.tensor.matmul(wps[0:128, 0:256], warm[:, 0:128], warm[:],
                                 start=True, stop=True)

            i1_sb = cpool.tile([H, 258 + K], F32R)
            wt_sb = i1_sb[:, 0:128]
            bcol_sb = i1_sb[:, 128:129].bitcast(F32)
            wt2_sb = i1_sb[:, 129:257]
            bcol2_sb = i1_sb[:, 257:258].bitcast(F32)
            dlgT_sb = i1_sb[:, 258:258 + K]
            xdT_sb = cpool.tile([H, B * LC], F32R)
            i3_sb = cpool.tile([128, 16 * 129], BF16)
            dlga_sb = i3_sb[:]  # (128, 2064)

            # DMAs enqueued in consumption order (HWDGE drains FIFO);
            # small leading pieces start the proj/scores train early.
            nc.sync.dma_start(i1_sb[:, 0:514], inp1[:, 0:514])
            nc.sync.dma_start(xdT_sb[:, 0:256], inp2[:, 0:256])
            nc.sync.dma_start(i1_sb[:, 514:770], inp1[:, 514:770])
            nc.sync.dma_start(xdT_sb[:, 256:512], inp2[:, 256:512])
            nc.sync.dma_start(i3_sb[:, 0:1032], inp3[:, 0:1032])
            nc.sync.dma_start(xdT_sb[:, 512:1024], inp2[:, 512:1024])
            nc.sync.dma_start(i1_sb[:, 770:1282], inp1[:, 770:1282])
            nc.sync.dma_start(xdT_sb[:, 1024:2048], inp2[:, 1024:2048])
            nc.sync.dma_start(i1_sb[:, 1282:2306], inp1[:, 1282:2306])
            nc.sync.dma_start(i3_sb[:, 1032:2064], inp3[:, 1032:2064])

            # projections: out[h, col] = relu(sum_d Wt[d, h] * inT[d, col] + b)
            yproj = cpool.tile([H, K], F32R)
            xproj = cpool.tile([H, B * LC], F32R)

            PIECES = [(0, 256), (256, 512), (512, 1024),
                      (1024, 1536), (1536, 2048)]

            def emit_proj(dst, src, lo, hi, name):
                scaled = (name == "y" and lo >= 512)
                wtile = wt2_sb if scaled else wt_sb
                btile = bcol2_sb if scaled else bcol_sb
                ps = psb.tile([128, STACK * 256], F32, tag="scps",
                              name=f"psproj_{name}{lo}")
                n = hi - lo
                nc.tensor.matmul(ps[:, 0:n], wtile, src[:, lo:hi],
                                 start=True, stop=True)
                if name == "y" and lo < 512:
                    # ACT is idle during the prologue; DVE handles the x
                    # side concurrently.
                    nc.scalar.activation(dst[:, lo:hi], ps[:, 0:n],
                                         mybir.ActivationFunctionType.Relu,
                                         bias=btile, scale=1.0)
                else:
                    nc.vector.tensor_scalar(dst[:, lo:hi], ps[:, 0:n],
                                            btile, 0.0,
                                            op0=mybir.AluOpType.add,
                                            op1=mybir.AluOpType.max)

            next_piece = {"y": 0, "x": 0}

            def need_proj(name, upto):
                dst, src = ((yproj, dlgT_sb) if name == "y"
                            else (xproj, xdT_sb))
                while next_piece[name] < len(PIECES) and \
                        PIECES[next_piece[name]][0] < upto:
                    lo, hi = PIECES[next_piece[name]]
                    emit_proj(dst, src, lo, hi, name)
                    next_piece[name] += 1

            need_proj("y", 256)
            need_proj("x", 256)

            # flat stack list with lookahead software pipeline:
            #   emit scores(i); process(i-LOOKAHEAD) = exp + memsets + out-MMs
            flat = []
            for g in ORDER:
                nchunks = _chunks_of_group(g)
                for s0 in range(0, nchunks, STACK):
                    flat.append((g, s0, min(STACK, nchunks - s0)))

            state = {}   # g -> psAB
            tiles = {}   # i -> (ps, pt)
            SCHRAUD_DVE = {8, 11, 13, 14, 15, 18}

            def emit_scores(i):
                g, s0, ns = flat[i]
                xg = xproj[:, g * 256:(g + 1) * 256]
                ps = psb.tile([128, STACK * 256], F32, tag="scps")
                pt = ptpool.tile([128, STACK * 256], BF16, tag="pt")
                tiles[i] = (ps, pt)
                for k in range(ns):
                    c = s0 + k
                    nc.tensor.matmul(ps[:, k * 256:(k + 1) * 256],
                                     yproj[:, c * 128:(c + 1) * 128], xg,
                                     start=True, stop=True)

            def emit_process(i):
                g, s0, ns = flat[i]
                nchunks = _chunks_of_group(g)
                ps, pt = tiles.pop(i)
                if s0 >= 4 and i in SCHRAUD_DVE:
                    # scores arrived A-scaled (Wt2 proj); custom op writes
                    # corrected-Schraudolph bf16 bits.
                    nc.vector._custom_dve(
                        EXP_OP, out=pt[:, 0:ns * 256].bitcast(I16),
                        in0=ps[:, 0:ns * 256],
                        s0=64.0, s1=M128, imm2=float(KAPPA))
                elif s0 >= 4:
                    # exact exp on ACT matching the scaled-chunk convention.
                    nc.scalar.activation(pt[:, 0:ns * 256], ps[:, 0:ns * 256],
                                         mybir.ActivationFunctionType.Exp,
                                         bias=negs2[:], scale=float(1.0 / A_C))
                else:
                    nc.scalar.activation(pt[:, 0:ns * 256], ps[:, 0:ns * 256],
                                         mybir.ActivationFunctionType.Exp,
                                         bias=negs0[:], scale=1.0)
                for k in range(ns):
                    c = s0 + k
                    blk = pt[:, k * 256:(k + 1) * 256]
                    if c == nchunks - 2:
                        nc.gpsimd.memset(blk[:, 0:64], 0)
                        nc.gpsimd.memset(blk[64:128, 64:128], 0)
                    elif c == nchunks - 1:
                        # cols 0:128 are never read (psA matmul skipped)
                        nc.gpsimd.memset(blk[:, 128:192], 0)
                        nc.gpsimd.memset(blk[64:128, 192:256], 0)
                if s0 == 0:
                    # psA/psB share one PSUM bank: psA's start=True clears the
                    # bank's has_written bits, so psB's first matmul must use
                    # start=False (overwrites the still-clear region).
                    psAB = pso.tile([128, 260], F32, tag="psout", name=f"ps{g}")
                    state[g] = psAB
                psAB = state[g]
                psA = psAB[:, 0:129]
                psB = psAB[:, 130:259]
                for k in range(ns):
                    c = s0 + k
                    dchunk = dlga_sb[:, c * 129:(c + 1) * 129]
                    blk = pt[:, k * 256:(k + 1) * 256]
                    if c < nchunks - 1:
                        # last chunk's cols 0:128 are causally all-zero: skip
                        nc.tensor.matmul(psA, blk[:, 0:128], dchunk,
                                         start=(c == 0),
                                         stop=(c == nchunks - 2))
                    nc.tensor.matmul(psB, blk[:, 128:256], dchunk,
                                     start=False, stop=(c == nchunks - 1))
                if s0 + ns == nchunks:
                    emit_flush(g)

            def emit_flush(g):
                psAB = state.pop(g)
                osb = osbpool.tile([128, 260], F32, tag="osb")
                if g == ORDER[-1]:
                    # final flush on ACT (free at the end; DVE still busy)
                    nc.scalar.activation(osb[:], psAB[:, 0:260],
                                         mybir.ActivationFunctionType.Copy,
                                         bias=0.0, scale=1.0)
                else:
                    nc.vector.tensor_copy(osb[:], psAB[:, 0:260])
                nc.sync.dma_start(out[g], osb[:])

            LOOKAHEAD = 6
            for i in range(len(flat) + LOOKAHEAD):
                if i < len(flat):
                    g, s0, ns = flat[i]
                    need_proj("y", 128 * (s0 + ns))
                    need_proj("x", 256 * (g + 1))
                    # emit the late proj pieces as soon as their DMA can be
                    # there: queueing them late stalls the whole back half.
                    if i == 5:
                        need_proj("x", 2048)
                    if i == 7:
                        need_proj("y", 2048)
                    emit_scores(i)
                j = i - LOOKAHEAD
                if 0 <= j < len(flat):
                    emit_process(j)

    nc.compile()
    return nc


def _get_nc():
    global _NC_CACHE
    if _NC_CACHE is None:
        _NC_CACHE = _build()
    return _NC_CACHE


def _round_f32r(a):
    u = np.ascontiguousarray(a, dtype=np.float32).view(np.uint32)
    r = ((u.astype(np.uint64) + 0x800) & 0xFFFFF000).astype(np.uint32)
    return r.view(np.float32)


LAST_RESULTS = None  # BassKernelResults of the most recent run (for test harness)


def kernel(xd_emb, xq_emb, xa_emb, W, b, recency_weight, xq_mask, xa_mask,
           _trace=False):
    xd_emb = np.asarray(xd_emb, np.float32)
    xq_emb = np.asarray(xq_emb, np.float32)
    xa_emb = np.asarray(xa_emb, np.float32)
    W = np.asarray(W, np.float32)
    b = np.asarray(b, np.float32)
    rw0 = float(np.asarray(recency_weight).reshape(-1)[0])
    pad = np.concatenate([np.asarray(xq_mask), np.asarray(xa_mask)], axis=1).reshape(K)

    dlg = np.concatenate([xq_emb, xa_emb], axis=1).reshape(K, H)
    ts = (np.arange(K) // M).astype(np.float64)
    phi = np.exp(-rw0 * (ts - T0))
    dlg_aug = np.concatenate([dlg.astype(np.float64), np.ones((K, 1))], axis=1)
    dlg_aug *= phi[:, None]
    # chunks >= 4 arrive as bits ~= A*s (no -A*S0 - 16256 bias): compensate
    dlg_aug[512:] *= np.exp(LOG_RDIV)
    dlg_aug[pad] = 0.0
    dlga_bf = dlg_aug.astype(ml_dtypes.bfloat16)
    dlga_packed = np.ascontiguousarray(
        dlga_bf.reshape(16, 128, 129).transpose(1, 0, 2).reshape(128, 16 * 129))

    inp1 = np.empty((H, 258 + K), np.float32)
    inp1[:, 0:128] = _round_f32r(W.T)
    inp1[:, 128] = b
    inp1[:, 129:257] = _round_f32r(np.float32(A_C) * W.T)
    inp1[:, 257] = np.float32(A_C) * b
    inp1[:, 258:] = _round_f32r(dlg.T)
    inp3 = dlga_packed  # (128, 2064) bf16

    xdT = xd_emb.transpose(2, 0, 1)  # (H, B, L1)
    in_maps = []
    for c in range(NCORES):
        xdT_c = xdT[:, :, c * LC:(c + 1) * LC].reshape(H, B * LC)
        in_maps.append({
            "inp1": inp1,
            "inp2": _round_f32r(xdT_c),
            "inp3": inp3,
        })

    nc = _get_nc()
    try:
        res = run_bass_kernel_spmd(nc, in_maps, list(range(NCORES)),
                                   trace=_trace)
    except ModuleNotFoundError:
        # The axon NTFF-profile hook is absent in this container; if an
        # ambient BASS_TRACE forced the trace path, retry without it.
        os.environ["BASS_NEVER_TRACE"] = "1"
        res = run_bass_kernel_spmd(nc, in_maps, list(range(NCORES)))
    global LAST_RESULTS
    LAST_RESULTS = res

    full = np.empty((B, L1, H), np.float64)
    for c in range(NCORES):
        raw = res.results[c]["out"].astype(np.float64)  # (8, 128, 260)
        for g in range(NG):
            numer = np.concatenate([raw[g, :, 0:128], raw[g, :, 130:258]],
                                   axis=0)                    # (256, 128)
            denom = np.concatenate([raw[g, :, 128], raw[g, :, 258]])[:, None]
            denom[denom == 0.0] = 1.0
            rows = numer / denom                              # (256, 128)
            for bb in range(4):
                full[4 * g + bb, c * LC:(c + 1) * LC] = \
                    rows[64 * bb:64 * (bb + 1)]
    full[0] = 0.0
    return np.ascontiguousarray(full, dtype=np.float32)
